# revision 28
# baseline (speedup 1.0000x reference)
"""Trainium2 Bass kernel for relu-kernelized multi-head attention with a
per-head Toeplitz relative-position mask (sparse_attention problem).

Contract: kernel(**inputs) takes FULL unsharded inputs (numpy), returns the
FULL output [16, 1025, 768]. Internally: data-parallel over batch across 8
NeuronCores (2 batches/core), identical SPMD program, per-core inputs differ
only in the x shard.

Math (per batch b):
  q = relu((x@wq + bq)/8) + eps ; k = relu(x@wk + bk) + eps ; v = x@wv + bv
  S[q,k] = sum_d q*k ;  attn = S*|tm| + eps ; attn /= rowsum ; out = attn@v
  y = out@wo + bo

Device-side layout choices:
  - everything feeding the PE array is bf16 (4x matmul throughput vs fp32,
    half the HBM traffic); PSUM accumulation stays fp32.  The correctness
    budget (2e-2 max-rel) dwarfs bf16 rounding.
  - x shipped transposed+padded with a ones-row: xaT [2, 769, 1152] so the
    QKV biases fold into the matmuls as a K=1 extra contraction chunk.
  - qT/kT produced in [head*64, token] layout -> S^T tiles [k,q] come
    straight from matmuls with K=d=64.  The reference's +eps on q/k is
    dropped: it perturbs S by ~1e-7 relative, far below the bf16 noise.
  - mask |tm| is gathered on host (it is pure input preprocessing: a
    Toeplitz-strided view of toeplitz_params), shipped transposed
    [h, k, q] in bf16, padded with zeros on the k dim.
  - v_aug [token, 65] per head carries a ones column: the AV matmul's row 64
    accumulates the rowsum for free.  The "+eps" of the reference rides in
    as a rank-1 matmul: eps * colsum(v_aug) outer ones.
  - normalization: 1/r = Exp(-Ln(r)) on the scalar engine (the DVE
    RECIPROCAL at 6.5us per [1,1025] row was blocking the mask-multiply
    pipeline and stalling the PE), gpsimd partition_broadcast to 64 rows,
    one vector multiply writing bf16 straight into the resident per-pair
    O^T tile (hh=1 lands on partitions 64..128).
  - no DRAM spill of O^T: the output projection reads the resident SBUF
    tiles; y^T = wo^T @ O^T + bo with the bias fused into the psum drain.
"""

import os
import sys

sys.path.insert(0, "/opt/trn_rl_repo")

import numpy as np

B, L, F, H, D = 16, 1025, 768, 12, 64
NB = 32
EPS = 1e-8
LP = 1152           # padded token count (9 * 128)
NKB = 9             # k blocks of 128
QM = 1024           # main q width (q tail = 1 col, index 1024)
FA = F + 1          # augmented contraction (ones row)
NCORES = 8
BPC = B // NCORES   # batches per core

_PROG = None


def _build_program():
    import concourse.bass as bass
    import concourse.tile as tile
    from concourse import mybir

    f32 = mybir.dt.float32
    bf16 = mybir.dt.bfloat16
    fp8 = mybir.dt.float8e4
    DR = mybir.MatmulPerfMode.DoubleRow
    AF = mybir.ActivationFunctionType

    nc = bass.Bass()

    xaT = nc.declare_dram_parameter("xaT", [BPC, FA, LP], bf16, isOutput=False)
    wq_aug = nc.declare_dram_parameter("wq_aug", [FA, F], bf16, isOutput=False)
    wk_aug = nc.declare_dram_parameter("wk_aug", [FA, F], bf16, isOutput=False)
    wv_aug = nc.declare_dram_parameter("wv_aug", [FA, H * 65], bf16, isOutput=False)
    wo_flat = nc.declare_dram_parameter("wo_flat", [H * D, F], bf16, isOutput=False)
    bo_in = nc.declare_dram_parameter("bo", [F], f32, isOutput=False)
    mask_main = nc.declare_dram_parameter(
        "maskT_main", [H, NKB, 128, QM], bf16, isOutput=False
    )
    mask_tail = nc.declare_dram_parameter(
        "maskT_tail", [H, 128, NKB], bf16, isOutput=False
    )
    yT = nc.declare_dram_parameter("yT", [BPC, F, L], f32, isOutput=True)

    rr_dram = nc.dram_tensor("rr_dram", [4, L], f32)
    bqk = nc.declare_dram_parameter("bqk_eff", [2, F], f32, isOutput=False)
    cs_in = nc.declare_dram_parameter("cs_cols", [BPC, 2, 65, 6], f32, isOutput=False)

    with tile.TileContext(nc) as tc:
        from contextlib import ExitStack

        with ExitStack() as ctx:
            consts = ctx.enter_context(tc.tile_pool(name="consts", bufs=1))
            xa_pool = ctx.enter_context(tc.tile_pool(name="xa", bufs=2 * 6))
            wqk_pool = ctx.enter_context(tc.tile_pool(name="wqk", bufs=2))
            wv_pool = ctx.enter_context(tc.tile_pool(name="wv", bufs=1))
            qkt_pool = ctx.enter_context(tc.tile_pool(name="qkt", bufs=2))
            qdr_pool = ctx.enter_context(tc.tile_pool(name="qdr", bufs=4))
            vaug_pool = ctx.enter_context(tc.tile_pool(name="vaug", bufs=2))
            csc_pool = ctx.enter_context(tc.tile_pool(name="cscol", bufs=2))
            bias_pool = ctx.enter_context(tc.tile_pool(name="bias", bufs=2))
            mask_pool = ctx.enter_context(tc.tile_pool(name="mask", bufs=12))
            mtail_pool = ctx.enter_context(tc.tile_pool(name="mtail", bufs=2))
            mt_pool = ctx.enter_context(tc.tile_pool(name="mt", bufs=3))
            mttail_pool = ctx.enter_context(tc.tile_pool(name="mttail", bufs=2))
            rr_pool = ctx.enter_context(tc.tile_pool(name="rr", bufs=2))
            rrb_pool = ctx.enter_context(tc.tile_pool(name="rrb", bufs=2))
            avsb_pool = ctx.enter_context(tc.tile_pool(name="avsb", bufs=2))
            ot_pool = ctx.enter_context(tc.tile_pool(name="ot", bufs=12))
            wo_pool = ctx.enter_context(tc.tile_pool(name="wo", bufs=6))
            y_pool = ctx.enter_context(tc.tile_pool(name="y", bufs=4))
            bo_pool = ctx.enter_context(tc.tile_pool(name="bo", bufs=1))

            ps_proj = ctx.enter_context(
                tc.tile_pool(name="ps_proj", bufs=2, space="PSUM")
            )
            ps_s = ctx.enter_context(tc.tile_pool(name="ps_s", bufs=2, space="PSUM"))
            ps_av = ctx.enter_context(tc.tile_pool(name="ps_av", bufs=1, space="PSUM"))

            dma = nc.sync

            # constants
            ones_row = consts.tile([1, LP], bf16)
            nc.vector.memset(ones_row[:, 0:L], 1.0)
            nc.vector.memset(ones_row[:, L:LP], 0.0)
            # q is NOT pre-scaled by 1/8 (better fp8 range; the sum-
            # normalization cancels the factor; eps consts absorb it on host)
            leps = consts.tile([128, 1], f32, name="leps")
            nc.vector.memset(leps, 8.0 * float(L) * EPS)

            # ---- persistent xaT in SBUF --------------------------------
            # 6 full 128-row chunks per batch + the ones-row (row 768).
            # batch 0 is loaded first so the first vproj can start while
            # batch 1 and the weights stream in behind it.
            xa = {}
            for b in range(BPC):
                for c in range(6):
                    xa[(b, c)] = xa_pool.tile(
                        [128, LP], bf16, tag="xa", name="xa_tile"
                    )
                xa[(b, 6)] = ones_row

            def load_xa(b):
                for c in range(6):
                    dma.dma_start(
                        out=xa[(b, c)], in_=xaT[b, c * 128 : (c + 1) * 128, :]
                    )

            load_xa(0)

            # q sub-tiles for projections (proj N<=512)
            qsubs = [(0, 512), (512, 512), (1024, 128)]
            # attention q tiling: main [0,1024) in 2 psum-bank halves + tail col
            def st_slices():
                return [(0, 512), (512, 512)]

            # ---- v projections + colsums, per 6-head group --------------
            # wv_aug columns are grouped per head: h*65 + (0..63 -> wv, 64 -> ones)
            vaug = {}      # (b, g) -> [128, NKB, 390]
            csum = {}      # (b, g) -> [65, 6]

            def load_wv(g):
                wv_sb = wv_pool.tile([128, 7, 390], bf16, tag="wv")
                c0 = g * 390
                for c in range(6):
                    dma.dma_start(
                        out=wv_sb[:, c, :],
                        in_=wv_aug[c * 128 : (c + 1) * 128, c0 : c0 + 390],
                    )
                dma.dma_start(
                    out=wv_sb[0:1, 6, :], in_=wv_aug[F : F + 1, c0 : c0 + 390]
                )
                return wv_sb

            def emit_vproj(g, wv_sb):
                for b in range(BPC):
                    va = vaug_pool.tile([128, NKB, 390], bf16, tag="vaug")
                    for tb in range(NKB):
                        ps = ps_proj.tile([128, 512], f32, tag="ps_p", name="ps_v")
                        for c in range(6):
                            nc.tensor.matmul(
                                ps[:, 0:390],
                                xa[(b, c)][:, tb * 128 : (tb + 1) * 128],
                                wv_sb[:, c, :],
                                start=(c == 0),
                                stop=False,
                            )
                        nc.tensor.matmul(
                            ps[:, 0:390],
                            xa[(b, 6)][:, tb * 128 : (tb + 1) * 128],
                            wv_sb[0:1, 6, :],
                            start=False,
                            stop=True,
                        )
                        nc.scalar.activation(va[:, tb, :], ps[:, 0:390], AF.Copy)
                    vaug[(b, g)] = va
                    cs_col = csc_pool.tile([65, 6], f32, tag="cscol")
                    dma.dma_start(out=cs_col, in_=cs_in[b, g])
                    csum[(b, g)] = cs_col

            # ---- main loop over head pairs ------------------------------
            ot_pair = {}   # (b, pair) -> [128, L] bf16 resident O^T
            wv_g0 = load_wv(0)
            load_xa(1)
            for pair in range(6):
                g = pair // 3
                if pair % 3 == 0:
                    emit_vproj(g, wv_g0 if g == 0 else load_wv(g))

                # qT/kT projections for this pair, both batches
                wq_sb = wqk_pool.tile([128, 6, 128], bf16, tag="wq")
                wk_sb = wqk_pool.tile([128, 6, 128], bf16, tag="wk")
                p0 = pair * 128
                for c in range(6):
                    dma.dma_start(
                        out=wq_sb[:, c, :],
                        in_=wq_aug[c * 128 : (c + 1) * 128, p0 : p0 + 128],
                    )
                    dma.dma_start(
                        out=wk_sb[:, c, :],
                        in_=wk_aug[c * 128 : (c + 1) * 128, p0 : p0 + 128],
                    )
                bq_sb = bias_pool.tile([128, 2], f32, tag="bqk")
                dma.dma_start(out=bq_sb[:, 0:1], in_=bqk[0, p0 : p0 + 128])
                dma.dma_start(out=bq_sb[:, 1:2], in_=bqk[1, p0 : p0 + 128])

                # qT/kT in fp8e4m3 [128, LP]; then a per-head 2-DMA repack
                # into the DoubleRow layout [32, 2, LP] (d 0..31 / 32..63
                # split into the free dim) for the 2x-rate S matmuls
                qdr = {}   # (b, hh) -> [32, 2, LP]
                kdr = {}
                for b in range(BPC):
                    qt = qkt_pool.tile([128, LP], fp8, tag="qT")
                    kt = qkt_pool.tile([128, LP], fp8, tag="kT")
                    for (dst, w_sb, bi) in ((qt, wq_sb, 0), (kt, wk_sb, 1)):
                        for (q0, qw) in qsubs:
                            psq = ps_proj.tile(
                                [128, 512], f32, tag="ps_p", name="ps_qk"
                            )
                            for c in range(6):
                                nc.tensor.matmul(
                                    psq[:, 0:qw],
                                    w_sb[:, c, :],
                                    xa[(b, c)][:, q0 : q0 + qw],
                                    start=(c == 0), stop=(c == 5),
                                )
                            # rel = relu(xw + b)   (+eps dropped)
                            nc.scalar.activation(
                                dst[:, q0 : q0 + qw], psq[:, 0:qw], AF.Relu,
                                scale=1.0, bias=bq_sb[:, bi : bi + 1],
                            )
                    for hh in range(2):
                        r0 = hh * 64
                        qd = qdr_pool.tile([32, 2, LP], fp8, tag="qdr")
                        kd = qdr_pool.tile([32, 2, LP], fp8, tag="kdr")
                        for (dr, src) in ((qd, qt), (kd, kt)):
                            dma.dma_start(
                                out=dr[:, 0, :], in_=src[r0 : r0 + 32, :]
                            )
                            dma.dma_start(
                                out=dr[:, 1, :], in_=src[r0 + 32 : r0 + 64, :]
                            )
                        qdr[(b, hh)] = qd
                        kdr[(b, hh)] = kd

                for b in range(BPC):
                    ot_pair[(b, pair)] = ot_pool.tile(
                        [128, L], bf16, tag="ot", name="ot_pair"
                    )

                for hh in range(2):
                    h = pair * 2 + hh
                    r0 = hh * 64
                    # mask tiles for this head (shared across batches)
                    mks = []
                    for j in range(NKB):
                        mk = mask_pool.tile([128, QM], bf16, tag="mask", name="mask_tile")
                        dma.dma_start(out=mk, in_=mask_main[h, j])
                        mks.append(mk)
                    mkt = mtail_pool.tile([128, NKB], bf16, tag="mtail")
                    dma.dma_start(out=mkt, in_=mask_tail[h])

                    for b in range(BPC):
                        va = vaug[(b, pair // 3)]
                        vc0 = (pair % 3) * 130 + hh * 65
                        cs = csum[(b, pair // 3)]

                        av = ps_av.tile([65, QM], f32, tag="ps_av")
                        ptl = ps_proj.tile([128, 512], f32, tag="ps_p", name="ps_tails")
                        stail = ptl[:, 0:NKB]
                        avt = ptl[0:65, NKB : NKB + 1]
                        mtt = mttail_pool.tile([128, NKB], bf16, tag="mttail")

                        qd = qdr[(b, hh)]
                        kd = kdr[(b, hh)]
                        for j in range(NKB):
                            st = ps_s.tile([128, QM], f32, tag="ps_s")
                            lhs_k = kd[:, :, j * 128 : (j + 1) * 128]
                            for (q0, qw) in st_slices():
                                nc.tensor.matmul(
                                    st[:, q0 : q0 + qw],
                                    lhs_k,
                                    qd[:, :, q0 : q0 + qw],
                                    start=True, stop=True,
                                    perf_mode=DR,
                                )
                            # tail column q=1024 (shares the kT weights)
                            nc.tensor.matmul(
                                stail[:, j : j + 1],
                                lhs_k,
                                qd[:, :, QM : QM + 1],
                                start=True, stop=True,
                                perf_mode=DR,
                            )
                            # masked scores (bf16 out feeds the AV matmul)
                            mt = mt_pool.tile([128, QM], bf16, tag="mt")
                            nc.vector.tensor_mul(mt, st, mks[j])
                            # AV accumulation (row 64 = rowsum via ones col)
                            for (q0, qw) in st_slices():
                                nc.tensor.matmul(
                                    av[:, q0 : q0 + qw],
                                    va[:, j, vc0 : vc0 + 65],
                                    mt[:, q0 : q0 + qw],
                                    start=(j == 0), stop=(j == NKB - 1),
                                )
                        # tail: masked scores + AV
                        nc.vector.tensor_mul(mtt, stail, mkt)
                        for j in range(NKB):
                            nc.tensor.matmul(
                                avt,
                                va[:, j, vc0 : vc0 + 65],
                                mtt[:, j : j + 1],
                                start=(j == 0), stop=(j == NKB - 1),
                            )

                        # drain AV psum to SBUF fast (frees the banks for
                        # the next head while the slow normalize chain runs)
                        av_sb = avsb_pool.tile([65, L], f32, tag="avsb")
                        nc.scalar.activation(av_sb[:, 0:512], av[:, 0:512], AF.Copy)
                        nc.scalar.activation(av_sb[:, 512:QM], av[:, 512:QM], AF.Copy)
                        nc.scalar.activation(av_sb[:, QM : QM + 1], avt, AF.Copy)
                        # normalization: row 64 = rowsum; 1/(r + L*eps) as
                        # Exp(-Ln(.)) on the ACT engine, then a gpsimd
                        # partition-broadcast to 64 rows
                        lnr = rr_pool.tile([1, L], f32, tag="rr")
                        nc.scalar.activation(
                            lnr, av_sb[64:65, :], AF.Ln, bias=leps[64:65, :]
                        )
                        rcp = rr_pool.tile([1, L], f32, tag="rcp")
                        nc.scalar.activation(rcp, lnr, AF.Exp, scale=-1.0)
                        rrb = rrb_pool.tile([64, L], f32, tag="rrb")
                        slot = (b * H + h) % 4
                        dma.dma_start(out=rr_dram[slot], in_=rcp)
                        rr_slot = rr_dram[slot]
                        rr_bcast_src = bass.AP(
                            tensor=rr_slot.tensor,
                            offset=rr_slot.offset,
                            ap=[[0, 64]] + list(rr_slot.ap),
                        )
                        dma.dma_start(out=rrb, in_=rr_bcast_src)
                        hg = (pair % 3) * 2 + hh
                        nc.vector.scalar_tensor_tensor(
                            ot_pair[(b, pair)][r0 : r0 + 64, :],
                            av_sb[0:64, :], cs[0:64, hg : hg + 1], rrb,
                            op0=mybir.AluOpType.add, op1=mybir.AluOpType.mult,
                        )

            # ---- output projection: yT = wo^T @ O^T + bo ----------------
            bo_sb = bo_pool.tile([128, 6], f32)
            for fc in range(6):
                dma.dma_start(
                    out=bo_sb[:, fc : fc + 1], in_=bo_in[fc * 128 : (fc + 1) * 128]
                )
            wo_sb = []
            for hc in range(6):
                t = wo_pool.tile([128, F], bf16, tag="wo", name="wo_tile")
                dma.dma_start(out=t, in_=wo_flat[hc * 128 : (hc + 1) * 128, :])
                wo_sb.append(t)

            oq_tiles = [(0, 512), (512, 512), (1024, 1)]
            for b in range(BPC):
                for (q0, qw) in oq_tiles:
                    for fc in range(6):
                        psy = ps_s.tile(
                            [128, 512], f32, tag="ps_s", name="ps_y"
                        )
                        for hc in range(6):
                            nc.tensor.matmul(
                                psy[:, 0:qw],
                                wo_sb[hc][:, fc * 128 : (fc + 1) * 128],
                                ot_pair[(b, hc)][:, q0 : q0 + qw],
                                start=(hc == 0), stop=(hc == 5),
                            )
                        ys = y_pool.tile([128, 512], f32, tag="y", name="y_tile")
                        nc.scalar.activation(
                            ys[:, 0:qw], psy[:, 0:qw], AF.Identity,
                            bias=bo_sb[:, fc : fc + 1],
                        )
                        dma.dma_start(
                            out=yT[b, fc * 128 : (fc + 1) * 128, q0 : q0 + qw],
                            in_=ys[:, 0:qw],
                        )

    _split_matmul_waits(nc)
    return nc


def _split_matmul_waits(nc):
    """Walrus TPB instruction structs encode a limited number of sync waits
    (the fp32 LDWEIGHTS+MATMUL pair can take none beyond its update).  Hoist
    excess waits onto same-engine NoOps inserted just before each
    instruction."""
    import bass_rust
    from concourse import mybir

    n = 0
    for f in nc.m.functions:
        for blk in f.blocks:
            insts = blk.instructions
            out = []
            for inst in insts:
                si = inst.sync_info
                tname = type(inst).__name__
                if (
                    si is not None
                    and len(si.on_wait) > 0
                    and "ISA" not in tname
                    and "CustomDve" not in tname
                ):
                    cap = 0 if tname == "InstMatmult" else 1
                    waits = list(si.on_wait)
                    if len(waits) > cap:
                        hoist = waits[: len(waits) - cap]
                        keep = waits[len(waits) - cap :]
                        for w in hoist:
                            nop = mybir.InstNoOp(
                                name=f"I-mmw-{n}", ins=[], outs=[]
                            )
                            n += 1
                            nop.engine = inst.engine
                            nop.sync_info = bass_rust.SyncInfo(
                                on_wait=[w], on_update=[]
                            )
                            out.append(nop)
                        inst.sync_info = bass_rust.SyncInfo(
                            on_wait=keep, on_update=list(si.on_update)
                        )
                out.append(inst)
            insts[:] = out
    return n


def _dist_index():
    gi = np.arange(NB)
    gj = np.arange(NB)
    idx = (
        (gi[:, None, None, None] - gi[None, None, :, None] + NB) * 2 * NB
        + gj[None, :, None, None]
        - gj[None, None, None, :]
        + NB
    )
    return idx.reshape(-1).astype(np.int32)


def _host_prep(x, wq, bq, wk, bk, wv, bv, wo, bo, toeplitz_params):
    import ml_dtypes

    f4 = np.float32
    bf = ml_dtypes.bfloat16
    x = np.asarray(x, f4)
    L0 = NB * NB

    xaT = np.zeros((B, FA, LP), bf)
    xaT[:, :F, :L] = np.transpose(x, (0, 2, 1)).astype(bf)
    xaT[:, F, :L] = 1.0

    wq_aug = np.empty((FA, F), f4)
    wq_aug[:F] = np.asarray(wq, f4).reshape(F, F)
    wq_aug[F] = np.asarray(bq, f4).reshape(F)
    wk_aug = np.empty((FA, F), f4)
    wk_aug[:F] = np.asarray(wk, f4).reshape(F, F)
    wk_aug[F] = np.asarray(bk, f4).reshape(F)

    wv_aug = np.zeros((FA, H * 65), f4)
    wvr = np.asarray(wv, f4)
    bvr = np.asarray(bv, f4)
    for h in range(H):
        wv_aug[:F, h * 65 : h * 65 + 64] = wvr[:, h, :]
        wv_aug[F, h * 65 : h * 65 + 64] = bvr[h]
        wv_aug[F, h * 65 + 64] = 1.0

    wo_flat = np.ascontiguousarray(np.asarray(wo, f4).reshape(H * D, F)).astype(bf)
    bo_arr = np.asarray(bo, f4).reshape(F)

    # gathered |toeplitz| mask, padded (CLS row/col of ones), transposed,
    # k padded to 1152 with zeros
    tp = np.asarray(toeplitz_params, f4)
    tm = np.abs(tp[:, _dist_index()]).reshape(H, L0, L0)
    tm_full = np.ones((H, L, L), f4)
    tm_full[:, 1:, 1:] = tm
    maskT = np.zeros((H, LP, L), bf)
    maskT[:, :L, :] = np.transpose(tm_full, (0, 2, 1)).astype(bf)
    maskT_main = np.ascontiguousarray(
        maskT[:, :, :QM].reshape(H, NKB, 128, QM)
    )
    maskT_tail = np.ascontiguousarray(
        maskT[:, :, QM].reshape(H, NKB, 128).transpose(0, 2, 1)
    )

    xsum = x[:, :, :].sum(axis=1)  # [B, F]
    cs = np.einsum("bf,fhd->bhd", xsum, wvr) + L * bvr[None]  # [B, H, 64]
    # x8: the device computes S without the reference's 1/8 q-scale (the
    # sum-normalization cancels it); the eps constants pick up the factor
    cs_full = np.concatenate(
        [cs, np.full((B, H, 1), float(L), np.float32)], axis=2
    ) * np.float32(8.0 * EPS)  # [B, H, 65]
    cs_cols = np.zeros((B, 2, 65, 6), f4)
    for g in range(2):
        for hh in range(6):
            cs_cols[:, g, :, hh] = cs_full[:, 6 * g + hh, :]
    bqk_eff = np.stack(
        [np.asarray(bq, f4).reshape(F), np.asarray(bk, f4).reshape(F)]
    )
    shared = dict(
        bqk_eff=bqk_eff,
        wq_aug=wq_aug.astype(bf),
        wk_aug=wk_aug.astype(bf),
        wv_aug=wv_aug.astype(bf),
        wo_flat=wo_flat,
        bo=bo_arr,
        maskT_main=maskT_main,
        maskT_tail=maskT_tail,
    )
    in_maps = []
    for c in range(NCORES):
        m = dict(shared)
        m["xaT"] = np.ascontiguousarray(xaT[c * BPC : (c + 1) * BPC])
        m["cs_cols"] = np.ascontiguousarray(cs_cols[c * BPC : (c + 1) * BPC])
        in_maps.append(m)
    return in_maps


def _get_program():
    global _PROG
    if _PROG is None:
        _PROG = _build_program()
    return _PROG


def run(trace=False, **inputs):
    from concourse.bass_utils import run_bass_kernel_spmd

    nc = _get_program()
    in_maps = _host_prep(**inputs)
    res = run_bass_kernel_spmd(nc, in_maps, list(range(NCORES)), trace=trace)
    outs = []
    for c in range(NCORES):
        yt = res.results[c]["yT"]  # [BPC, F, L]
        outs.append(np.transpose(yt, (0, 2, 1)))
    y = np.concatenate(outs, axis=0).astype(np.float32)
    return y, res


def kernel(**inputs):
    y, _ = run(trace=False, **inputs)
    return y


# revision 40
# speedup vs baseline: 1.3158x; 1.3158x over previous
"""Trainium2 Bass kernel for relu-kernelized multi-head attention with a
per-head Toeplitz relative-position mask (sparse_attention problem).

Contract: kernel(**inputs) takes FULL unsharded inputs (numpy), returns the
FULL output [16, 1025, 768]. Internally: data-parallel over batch across 8
NeuronCores (2 batches/core), identical SPMD program, per-core inputs differ
only in the x shard.

Math (per batch b):
  q = relu((x@wq + bq)/8) + eps ; k = relu(x@wk + bk) + eps ; v = x@wv + bv
  S[q,k] = sum_d q*k ;  attn = S*|tm| + eps ; attn /= rowsum ; out = attn@v
  y = out@wo + bo

Device-side layout choices:
  - everything feeding the PE array is bf16 (4x matmul throughput vs fp32,
    half the HBM traffic); PSUM accumulation stays fp32.  The correctness
    budget (2e-2 max-rel) dwarfs bf16 rounding.
  - x shipped transposed+padded with a ones-row: xaT [2, 769, 1152] so the
    QKV biases fold into the matmuls as a K=1 extra contraction chunk.
  - qT/kT produced in [head*64, token] layout -> S^T tiles [k,q] come
    straight from matmuls with K=d=64.  The reference's +eps on q/k is
    dropped: it perturbs S by ~1e-7 relative, far below the bf16 noise.
  - mask |tm| is gathered on host (it is pure input preprocessing: a
    Toeplitz-strided view of toeplitz_params), shipped transposed
    [h, k, q] in bf16, padded with zeros on the k dim.
  - v_aug [token, 65] per head carries a ones column: the AV matmul's row 64
    accumulates the rowsum for free.  The "+eps" of the reference rides in
    as a rank-1 matmul: eps * colsum(v_aug) outer ones.
  - normalization: 1/r = Exp(-Ln(r)) on the scalar engine (the DVE
    RECIPROCAL at 6.5us per [1,1025] row was blocking the mask-multiply
    pipeline and stalling the PE), gpsimd partition_broadcast to 64 rows,
    one vector multiply writing bf16 straight into the resident per-pair
    O^T tile (hh=1 lands on partitions 64..128).
  - no DRAM spill of O^T: the output projection reads the resident SBUF
    tiles; y^T = wo^T @ O^T + bo with the bias fused into the psum drain.
"""

import os
import sys

sys.path.insert(0, "/opt/trn_rl_repo")

import numpy as np

B, L, F, H, D = 16, 1025, 768, 12, 64
NB = 32
EPS = 1e-8
LP = 1152           # padded token count (9 * 128)
NKB = 9             # k blocks of 128
QM = 1024           # main q width (q tail = 1 col, index 1024)
FA = F + 1          # augmented contraction (ones row)
NCORES = 8
BPC = B // NCORES   # batches per core

_PROG = None


def _build_program():
    import concourse.bass as bass
    import concourse.tile as tile
    from concourse import mybir

    f32 = mybir.dt.float32
    bf16 = mybir.dt.bfloat16
    AF = mybir.ActivationFunctionType

    nc = bass.Bass()

    xaT = nc.declare_dram_parameter("xaT", [BPC, FA, LP], bf16, isOutput=False)
    wq_aug = nc.declare_dram_parameter("wq_aug", [FA, F], bf16, isOutput=False)
    wk_aug = nc.declare_dram_parameter("wk_aug", [FA, F], bf16, isOutput=False)
    wv_aug = nc.declare_dram_parameter("wv_aug", [FA, H * 65], bf16, isOutput=False)
    wo_flat = nc.declare_dram_parameter("wo_flat", [H * D, F], bf16, isOutput=False)
    bo_in = nc.declare_dram_parameter("bo", [F], f32, isOutput=False)
    mask_main = nc.declare_dram_parameter(
        "maskT_main", [H, 128, NKB, QM], bf16, isOutput=False
    )
    mask_tail = nc.declare_dram_parameter(
        "maskT_tail", [H, 128, NKB], bf16, isOutput=False
    )
    yT = nc.declare_dram_parameter("yT", [BPC, F, L], f32, isOutput=True)

    rr_dram = nc.dram_tensor("rr_dram", [4, L], f32)
    bqk = nc.declare_dram_parameter("bqk_eff", [2, F], f32, isOutput=False)
    cs_in = nc.declare_dram_parameter("cs_cols", [BPC, 2, 65, 6], f32, isOutput=False)

    with tile.TileContext(nc) as tc:
        from contextlib import ExitStack

        with ExitStack() as ctx:
            consts = ctx.enter_context(tc.tile_pool(name="consts", bufs=1))
            xa_pool = ctx.enter_context(tc.tile_pool(name="xa", bufs=2 * 6))
            wqk_pool = ctx.enter_context(tc.tile_pool(name="wqk", bufs=2))
            wv_pool = ctx.enter_context(tc.tile_pool(name="wv", bufs=1))
            qkt_pool = ctx.enter_context(tc.tile_pool(name="qkt", bufs=2))
            vaug_pool = ctx.enter_context(tc.tile_pool(name="vaug", bufs=2))
            csc_pool = ctx.enter_context(tc.tile_pool(name="cscol", bufs=2))
            bias_pool = ctx.enter_context(tc.tile_pool(name="bias", bufs=2))
            mask_pool = ctx.enter_context(tc.tile_pool(name="mask", bufs=2))
            mtail_pool = ctx.enter_context(tc.tile_pool(name="mtail", bufs=2))
            mt_pool = ctx.enter_context(tc.tile_pool(name="mt", bufs=3))
            mttail_pool = ctx.enter_context(tc.tile_pool(name="mttail", bufs=2))
            rr_pool = ctx.enter_context(tc.tile_pool(name="rr", bufs=2))
            rrb_pool = ctx.enter_context(tc.tile_pool(name="rrb", bufs=2))
            avsb_pool = ctx.enter_context(tc.tile_pool(name="avsb", bufs=2))
            ot_pool = ctx.enter_context(tc.tile_pool(name="ot", bufs=12))
            wo_pool = ctx.enter_context(tc.tile_pool(name="wo", bufs=6))
            y_pool = ctx.enter_context(tc.tile_pool(name="y", bufs=4))
            bo_pool = ctx.enter_context(tc.tile_pool(name="bo", bufs=1))

            ps_proj = ctx.enter_context(
                tc.tile_pool(name="ps_proj", bufs=2, space="PSUM")
            )
            ps_s = ctx.enter_context(tc.tile_pool(name="ps_s", bufs=2, space="PSUM"))
            ps_av = ctx.enter_context(tc.tile_pool(name="ps_av", bufs=1, space="PSUM"))

            dma = nc.sync

            # constants
            ones_row = consts.tile([1, LP], bf16)
            nc.vector.memset(ones_row[:, 0:L], 1.0)
            nc.vector.memset(ones_row[:, L:LP], 0.0)
            # q is NOT pre-scaled by 1/8 (better fp8 range; the sum-
            # normalization cancels the factor; eps consts absorb it on host)
            leps = consts.tile([128, 1], f32, name="leps")
            nc.vector.memset(leps, 8.0 * float(L) * EPS)

            # ---- persistent xaT in SBUF --------------------------------
            # 6 full 128-row chunks per batch + the ones-row (row 768).
            # batch 0 is loaded first so the first vproj can start while
            # batch 1 and the weights stream in behind it.
            xa = {}
            for b in range(BPC):
                for c in range(6):
                    xa[(b, c)] = xa_pool.tile(
                        [128, LP], bf16, tag="xa", name="xa_tile"
                    )
                xa[(b, 6)] = ones_row

            def load_xa(b):
                for c in range(6):
                    dma.dma_start(
                        out=xa[(b, c)], in_=xaT[b, c * 128 : (c + 1) * 128, :]
                    )

            load_xa(0)

            # q sub-tiles for projections (proj N<=512)
            qsubs = [(0, 512), (512, 512), (1024, 128)]
            # attention q tiling: main [0,1024) in 2 psum-bank halves + tail col
            def st_slices():
                return [(0, 512), (512, 512)]

            # ---- v projections + colsums, per 6-head group --------------
            # wv_aug columns are grouped per head: h*65 + (0..63 -> wv, 64 -> ones)
            vaug = {}      # (b, g) -> [128, NKB, 390]
            csum = {}      # (b, g) -> [65, 6]

            def load_wv(g):
                wv_sb = wv_pool.tile([128, 7, 390], bf16, tag="wv")
                c0 = g * 390
                for c in range(6):
                    dma.dma_start(
                        out=wv_sb[:, c, :],
                        in_=wv_aug[c * 128 : (c + 1) * 128, c0 : c0 + 390],
                    )
                dma.dma_start(
                    out=wv_sb[0:1, 6, :], in_=wv_aug[F : F + 1, c0 : c0 + 390]
                )
                return wv_sb

            def emit_vproj(g, wv_sb):
                for b in range(BPC):
                    va = vaug_pool.tile([128, NKB, 390], bf16, tag="vaug")
                    for tb in range(NKB):
                        ps = ps_proj.tile([128, 512], f32, tag="ps_p", name="ps_v")
                        for c in range(6):
                            nc.tensor.matmul(
                                ps[:, 0:390],
                                xa[(b, c)][:, tb * 128 : (tb + 1) * 128],
                                wv_sb[:, c, :],
                                start=(c == 0),
                                stop=False,
                            )
                        nc.tensor.matmul(
                            ps[:, 0:390],
                            xa[(b, 6)][:, tb * 128 : (tb + 1) * 128],
                            wv_sb[0:1, 6, :],
                            start=False,
                            stop=True,
                        )
                        nc.scalar.activation(va[:, tb, :], ps[:, 0:390], AF.Copy)
                    vaug[(b, g)] = va
                    cs_col = csc_pool.tile([65, 6], f32, tag="cscol")
                    dma.dma_start(out=cs_col, in_=cs_in[b, g])
                    csum[(b, g)] = cs_col

            # ---- main loop over head pairs ------------------------------
            ot_pair = {}   # pair -> [128, BPC, L] bf16 resident O^T
            wv_g0 = load_wv(0)
            load_xa(1)
            for pair in range(6):
                g = pair // 3
                if pair % 3 == 0:
                    emit_vproj(g, wv_g0 if g == 0 else load_wv(g))

                # qT/kT projections for this pair, both batches
                wq_sb = wqk_pool.tile([128, 6, 128], bf16, tag="wq")
                wk_sb = wqk_pool.tile([128, 6, 128], bf16, tag="wk")
                p0 = pair * 128
                for c in range(6):
                    dma.dma_start(
                        out=wq_sb[:, c, :],
                        in_=wq_aug[c * 128 : (c + 1) * 128, p0 : p0 + 128],
                    )
                    dma.dma_start(
                        out=wk_sb[:, c, :],
                        in_=wk_aug[c * 128 : (c + 1) * 128, p0 : p0 + 128],
                    )
                bq_sb = bias_pool.tile([128, 2], f32, tag="bqk")
                dma.dma_start(out=bq_sb[:, 0:1], in_=bqk[0, p0 : p0 + 128])
                dma.dma_start(out=bq_sb[:, 1:2], in_=bqk[1, p0 : p0 + 128])

                qT = {}
                kT = {}
                for b in range(BPC):
                    qt = qkt_pool.tile([128, LP], bf16, tag="qT")
                    kt = qkt_pool.tile([128, LP], bf16, tag="kT")
                    for (dst, w_sb, bi) in ((qt, wq_sb, 0), (kt, wk_sb, 1)):
                        for (q0, qw) in qsubs:
                            psq = ps_proj.tile(
                                [128, 512], f32, tag="ps_p", name="ps_qk"
                            )
                            for c in range(6):
                                nc.tensor.matmul(
                                    psq[:, 0:qw],
                                    w_sb[:, c, :],
                                    xa[(b, c)][:, q0 : q0 + qw],
                                    start=(c == 0), stop=(c == 5),
                                )
                            # rel = relu(xw + b)   (+eps dropped)
                            nc.scalar.activation(
                                dst[:, q0 : q0 + qw], psq[:, 0:qw], AF.Relu,
                                scale=1.0, bias=bq_sb[:, bi : bi + 1],
                            )
                    qT[b] = qt
                    kT[b] = kt

                ot_pair[pair] = ot_pool.tile(
                    [128, BPC, L], bf16, tag="ot", name="ot_pair"
                )

                for hh in range(2):
                    h = pair * 2 + hh
                    r0 = hh * 64
                    # whole-head mask in one 2.1MB DMA (18KB contiguous per
                    # partition); ring of 2 prefetches one head ahead
                    mk = mask_pool.tile(
                        [128, NKB, QM], bf16, tag="mask", name="mask_tile"
                    )
                    dma.dma_start(out=mk, in_=mask_main[h])
                    mks = [mk[:, j, :] for j in range(NKB)]
                    mkt = mtail_pool.tile([128, NKB], bf16, tag="mtail")
                    dma.dma_start(out=mkt, in_=mask_tail[h])

                    for b in range(BPC):
                        va = vaug[(b, pair // 3)]
                        vc0 = (pair % 3) * 130 + hh * 65
                        cs = csum[(b, pair // 3)]

                        av = ps_av.tile([65, QM], f32, tag="ps_av")
                        ptl = ps_proj.tile([128, 512], f32, tag="ps_p", name="ps_tails")
                        stail = ptl[:, 0:NKB]
                        avt = ptl[0:65, NKB : NKB + 1]
                        mtt = mttail_pool.tile([128, NKB], bf16, tag="mttail")

                        for j in range(NKB):
                            st = ps_s.tile([128, QM], f32, tag="ps_s")
                            lhs_k = kT[b][r0 : r0 + 64, j * 128 : (j + 1) * 128]
                            for (q0, qw) in st_slices():
                                nc.tensor.matmul(
                                    st[:, q0 : q0 + qw],
                                    lhs_k,
                                    qT[b][r0 : r0 + 64, q0 : q0 + qw],
                                    start=True, stop=True,
                                )
                            # tail column q=1024 (shares the kT weights)
                            nc.tensor.matmul(
                                stail[:, j : j + 1],
                                lhs_k,
                                qT[b][r0 : r0 + 64, QM : QM + 1],
                                start=True, stop=True,
                            )
                            # masked scores (bf16 out feeds the AV matmul)
                            mt = mt_pool.tile([128, QM], bf16, tag="mt")
                            nc.vector.tensor_mul(mt, st, mks[j])
                            # AV accumulation (row 64 = rowsum via ones col)
                            for (q0, qw) in st_slices():
                                nc.tensor.matmul(
                                    av[:, q0 : q0 + qw],
                                    va[:, j, vc0 : vc0 + 65],
                                    mt[:, q0 : q0 + qw],
                                    start=(j == 0), stop=(j == NKB - 1),
                                )
                        # tail: masked scores + AV
                        nc.vector.tensor_mul(mtt, stail, mkt)
                        for j in range(NKB):
                            nc.tensor.matmul(
                                avt,
                                va[:, j, vc0 : vc0 + 65],
                                mtt[:, j : j + 1],
                                start=(j == 0), stop=(j == NKB - 1),
                            )

                        # drain AV psum to SBUF fast (frees the banks for
                        # the next head while the slow normalize chain runs)
                        av_sb = avsb_pool.tile([65, L], f32, tag="avsb")
                        nc.scalar.activation(av_sb[:, 0:512], av[:, 0:512], AF.Copy)
                        nc.scalar.activation(av_sb[:, 512:QM], av[:, 512:QM], AF.Copy)
                        nc.scalar.activation(av_sb[:, QM : QM + 1], avt, AF.Copy)
                        # normalization: row 64 = rowsum; 1/(r + L*eps) as
                        # Exp(-Ln(.)) on the ACT engine, then a gpsimd
                        # partition-broadcast to 64 rows
                        lnr = rr_pool.tile([1, L], f32, tag="rr")
                        nc.scalar.activation(
                            lnr, av_sb[64:65, :], AF.Ln, bias=leps[64:65, :]
                        )
                        rcp = rr_pool.tile([1, L], f32, tag="rcp")
                        nc.scalar.activation(rcp, lnr, AF.Exp, scale=-1.0)
                        rrb = rrb_pool.tile([64, L], f32, tag="rrb")
                        slot = (b * H + h) % 4
                        dma.dma_start(out=rr_dram[slot], in_=rcp)
                        rr_slot = rr_dram[slot]
                        rr_bcast_src = bass.AP(
                            tensor=rr_slot.tensor,
                            offset=rr_slot.offset,
                            ap=[[0, 64]] + list(rr_slot.ap),
                        )
                        dma.dma_start(out=rrb, in_=rr_bcast_src)
                        hg = (pair % 3) * 2 + hh
                        nc.vector.scalar_tensor_tensor(
                            ot_pair[pair][r0 : r0 + 64, b, :],
                            av_sb[0:64, :], cs[0:64, hg : hg + 1], rrb,
                            op0=mybir.AluOpType.add, op1=mybir.AluOpType.mult,
                        )

            # ---- output projection: yT = wo^T @ O^T + bo ----------------
            bo_sb = bo_pool.tile([128, 6], f32)
            for fc in range(6):
                dma.dma_start(
                    out=bo_sb[:, fc : fc + 1], in_=bo_in[fc * 128 : (fc + 1) * 128]
                )
            wo_sb = []
            for hc in range(6):
                t = wo_pool.tile([128, F], bf16, tag="wo", name="wo_tile")
                dma.dma_start(out=t, in_=wo_flat[hc * 128 : (hc + 1) * 128, :])
                wo_sb.append(t)

            oq_tiles = [(0, 512), (512, 512)]
            for b in range(BPC):
                for (q0, qw) in oq_tiles:
                    for fc in range(6):
                        psy = ps_s.tile(
                            [128, 512], f32, tag="ps_s", name="ps_y"
                        )
                        for hc in range(6):
                            nc.tensor.matmul(
                                psy[:, 0:qw],
                                wo_sb[hc][:, fc * 128 : (fc + 1) * 128],
                                ot_pair[hc][:, b, q0 : q0 + qw],
                                start=(hc == 0), stop=(hc == 5),
                            )
                        ys = y_pool.tile([128, 512], f32, tag="y", name="y_tile")
                        nc.scalar.activation(
                            ys[:, 0:qw], psy[:, 0:qw], AF.Identity,
                            bias=bo_sb[:, fc : fc + 1],
                        )
                        dma.dma_start(
                            out=yT[b, fc * 128 : (fc + 1) * 128, q0 : q0 + qw],
                            in_=ys[:, 0:qw],
                        )
            # q=1024 tail column, both batches in one N=2 matmul per fc
            for fc in range(6):
                psy = ps_s.tile([128, 512], f32, tag="ps_s", name="ps_yt")
                for hc in range(6):
                    nc.tensor.matmul(
                        psy[:, 0:BPC],
                        wo_sb[hc][:, fc * 128 : (fc + 1) * 128],
                        ot_pair[hc][:, :, QM : QM + 1],
                        start=(hc == 0), stop=(hc == 5),
                    )
                ys = y_pool.tile([128, 512], f32, tag="y", name="y_tail")
                nc.scalar.activation(
                    ys[:, 0:BPC], psy[:, 0:BPC], AF.Identity,
                    bias=bo_sb[:, fc : fc + 1],
                )
                for b in range(BPC):
                    dma.dma_start(
                        out=yT[b, fc * 128 : (fc + 1) * 128, QM : QM + 1],
                        in_=ys[:, b : b + 1],
                    )

    _split_matmul_waits(nc)
    return nc


def _split_matmul_waits(nc):
    """Walrus TPB instruction structs encode a limited number of sync waits
    (the fp32 LDWEIGHTS+MATMUL pair can take none beyond its update).  Hoist
    excess waits onto same-engine NoOps inserted just before each
    instruction."""
    import bass_rust
    from concourse import mybir

    n = 0
    for f in nc.m.functions:
        for blk in f.blocks:
            insts = blk.instructions
            out = []
            for inst in insts:
                si = inst.sync_info
                tname = type(inst).__name__
                if (
                    si is not None
                    and len(si.on_wait) > 0
                    and "ISA" not in tname
                    and "CustomDve" not in tname
                ):
                    cap = 0 if tname == "InstMatmult" else 1
                    waits = list(si.on_wait)
                    if len(waits) > cap:
                        hoist = waits[: len(waits) - cap]
                        keep = waits[len(waits) - cap :]
                        for w in hoist:
                            nop = mybir.InstNoOp(
                                name=f"I-mmw-{n}", ins=[], outs=[]
                            )
                            n += 1
                            nop.engine = inst.engine
                            nop.sync_info = bass_rust.SyncInfo(
                                on_wait=[w], on_update=[]
                            )
                            out.append(nop)
                        inst.sync_info = bass_rust.SyncInfo(
                            on_wait=keep, on_update=list(si.on_update)
                        )
                out.append(inst)
            insts[:] = out
    return n


def _dist_index():
    gi = np.arange(NB)
    gj = np.arange(NB)
    idx = (
        (gi[:, None, None, None] - gi[None, None, :, None] + NB) * 2 * NB
        + gj[None, :, None, None]
        - gj[None, None, None, :]
        + NB
    )
    return idx.reshape(-1).astype(np.int32)


def _host_prep(x, wq, bq, wk, bk, wv, bv, wo, bo, toeplitz_params):
    import ml_dtypes

    f4 = np.float32
    bf = ml_dtypes.bfloat16
    x = np.asarray(x, f4)
    L0 = NB * NB

    xaT = np.zeros((B, FA, LP), bf)
    xaT[:, :F, :L] = np.transpose(x, (0, 2, 1)).astype(bf)
    xaT[:, F, :L] = 1.0

    wq_aug = np.empty((FA, F), f4)
    wq_aug[:F] = np.asarray(wq, f4).reshape(F, F)
    wq_aug[F] = np.asarray(bq, f4).reshape(F)
    wk_aug = np.empty((FA, F), f4)
    wk_aug[:F] = np.asarray(wk, f4).reshape(F, F)
    wk_aug[F] = np.asarray(bk, f4).reshape(F)

    wv_aug = np.zeros((FA, H * 65), f4)
    wvr = np.asarray(wv, f4)
    bvr = np.asarray(bv, f4)
    for h in range(H):
        wv_aug[:F, h * 65 : h * 65 + 64] = wvr[:, h, :]
        wv_aug[F, h * 65 : h * 65 + 64] = bvr[h]
        wv_aug[F, h * 65 + 64] = 1.0

    wo_flat = np.ascontiguousarray(np.asarray(wo, f4).reshape(H * D, F)).astype(bf)
    bo_arr = np.asarray(bo, f4).reshape(F)

    # gathered |toeplitz| mask, padded (CLS row/col of ones), transposed,
    # k padded to 1152 with zeros
    tp = np.asarray(toeplitz_params, f4)
    tm = np.abs(tp[:, _dist_index()]).reshape(H, L0, L0)
    tm_full = np.ones((H, L, L), f4)
    tm_full[:, 1:, 1:] = tm
    maskT = np.zeros((H, LP, L), bf)
    maskT[:, :L, :] = np.transpose(tm_full, (0, 2, 1)).astype(bf)
    # [H, 128, NKB, QM]: partition-major so one DMA per head is contiguous
    maskT_main = np.ascontiguousarray(
        maskT[:, :, :QM].reshape(H, NKB, 128, QM).transpose(0, 2, 1, 3)
    )
    maskT_tail = np.ascontiguousarray(
        maskT[:, :, QM].reshape(H, NKB, 128).transpose(0, 2, 1)
    )

    xsum = x[:, :, :].sum(axis=1)  # [B, F]
    cs = np.einsum("bf,fhd->bhd", xsum, wvr) + L * bvr[None]  # [B, H, 64]
    # x8: the device computes S without the reference's 1/8 q-scale (the
    # sum-normalization cancels it); the eps constants pick up the factor
    cs_full = np.concatenate(
        [cs, np.full((B, H, 1), float(L), np.float32)], axis=2
    ) * np.float32(8.0 * EPS)  # [B, H, 65]
    cs_cols = np.zeros((B, 2, 65, 6), f4)
    for g in range(2):
        for hh in range(6):
            cs_cols[:, g, :, hh] = cs_full[:, 6 * g + hh, :]
    bqk_eff = np.stack(
        [np.asarray(bq, f4).reshape(F), np.asarray(bk, f4).reshape(F)]
    )
    shared = dict(
        bqk_eff=bqk_eff,
        wq_aug=wq_aug.astype(bf),
        wk_aug=wk_aug.astype(bf),
        wv_aug=wv_aug.astype(bf),
        wo_flat=wo_flat,
        bo=bo_arr,
        maskT_main=maskT_main,
        maskT_tail=maskT_tail,
    )
    in_maps = []
    for c in range(NCORES):
        m = dict(shared)
        m["xaT"] = np.ascontiguousarray(xaT[c * BPC : (c + 1) * BPC])
        m["cs_cols"] = np.ascontiguousarray(cs_cols[c * BPC : (c + 1) * BPC])
        in_maps.append(m)
    return in_maps


def _get_program():
    global _PROG
    if _PROG is None:
        _PROG = _build_program()
    return _PROG


def run(trace=False, **inputs):
    from concourse.bass_utils import run_bass_kernel_spmd

    nc = _get_program()
    in_maps = _host_prep(**inputs)
    res = run_bass_kernel_spmd(nc, in_maps, list(range(NCORES)), trace=trace)
    outs = []
    for c in range(NCORES):
        yt = res.results[c]["yT"]  # [BPC, F, L]
        outs.append(np.transpose(yt, (0, 2, 1)))
    y = np.concatenate(outs, axis=0).astype(np.float32)
    return y, res


def kernel(**inputs):
    y, _ = run(trace=False, **inputs)
    return y
